# revision 2
# baseline (speedup 1.0000x reference)
"""Trainium2 Bass kernel for dense multi-head self-attention.

Reference computation (fp32):
    xn  = rms_norm(x) * (gamma + 1)          # F.normalize(x) * sqrt(D) * (gamma+1)
    qkv = xn @ w_qkv ; split into q, k, v    # heads H=16, dim_head 64
    out = softmax(q k^T / sqrt(64)) v
    y   = out @ w_out
Sharding (8 cores): data-parallel over batch (2), tensor-parallel over heads
(16 -> 4 groups of 4). Core c handles batch c//4, head group c%4. w_qkv is
column-sliced, w_out row-sliced per head group; each core emits a partial
[2048, 1024] output which the host sums per batch. No cross-device
communication inside the kernel.

v4 (builds on v3's 226us all-bf16 design; fp8 was measured and rejected:
elementwise fp8 noise passes straight through the random-walk attention
sums -> 4.7%+ absmax vs the 2% budget):
  - exp ops merged: ONE activation / tensor_scalar per (kcp, half) covering
    BOTH subs ([128, 2, 512] psum -> strided est write), halving ACT/DVE
    instruction count and semaphore traffic. st psum tiles are [128, 2, 512]
    (2 banks); sub0/sub1 score matmuls fill the two banks.
  - denominator reciprocals use reciprocal_approx_fast (~5x faster, 18-bit).
  - pipelined normalization multiplies move to GPSIMD (8% busy) right after
    its partition_broadcast; the last query group keeps DVE mults for the
    shortest tail chain.
  - head: weights stream on the ACT queue, all x halves on the SP queue; the
    first 512-token q/k projection chains are emitted split by dim-halves so
    the PE starts ~1.4us in and never gaps long enough to re-throttle HAM.
  - output stores merged to one [128, 1024] DMA per row block; norm DMAs on
    the SP queue; ACT engine keeps only its exp/copy work.
"""

import numpy as np

import concourse.bass as bass  # noqa: F401
import concourse.mybir as mybir
import concourse.tile as tile
from concourse import bacc
from concourse.bass_utils import run_bass_kernel_spmd

# Problem constants (hardcoded per contract; kernel.py must be self-contained).
B = 2          # batch
N = 2048       # sequence length
D = 1024       # model dim
H = 16         # total heads
DH = 64        # dim per head
HL = 4         # heads per core
DQ = HL * DH   # 256 = per-core q/k/v width
NCORES = 8

P = 128        # partitions

F32 = mybir.dt.float32
BF16 = mybir.dt.bfloat16
I16 = mybir.dt.int16

# Schraudolph fast-exp constants in bf16-exponent space:
#   bits = s * (2^7 / ln 2) + (127*2^7 - c);  bitcast<bf16>(bits) ~ exp(s)
# c = 2^7 * 0.043 balances the (1+f)/2^f linear-interp error to +-3%.
EXP_A = 128.0 / np.log(2.0)
EXP_B = 16256.0 - 5.513


def build_attention_kernel_v4(n=N, d=D, hl=HL, dh=DH):
    """Build the single-core SPMD Bass program (v4, all-bf16)."""
    PDT = BF16
    ADT = BF16
    dq = hl * dh
    ndc = d // P        # dim chunks of 128
    nt4 = n // 512      # token tiles of 512
    nt16 = n // P       # token tiles of 128
    kc_n = n // P       # key chunks of 128
    qg_n = n // 512     # query groups of 512
    hp_n = hl // 2      # head pairs
    ov_delay = 6

    # Routing of the 16 (kcp, half) exp units per (qg, hp) to DVE: strict
    # half-alternation (half0 -> ACT, half1 -> DVE) except u=11 -> ACT, where
    # the per-qg DVE reciprocal is scheduled so it never delays a fast-exp.
    dve_unit = [(u % 2 == 1) and u != 11 for u in range(16)]

    nc = bacc.Bacc()
    # xT arrives already rms-normalized (host folds rs[t] = sqrt(d)/||x_t||
    # into the columns). All inputs are HOST-PRE-TILED to the on-chip layout
    # so every DMA reads long contiguous runs.
    n_halves = 4 if n >= 2048 else (2 if n >= 1024 else 1)
    nh = n // n_halves
    xT_d = nc.declare_dram_parameter(
        "xT", [n_halves, P, d // P, nh], PDT, isOutput=False
    )
    wqkv_d = nc.declare_dram_parameter(
        "wqkv", [P, d // P, 3 * dq], PDT, isOutput=False
    )
    wout_d = nc.declare_dram_parameter(
        "wout", [P, dq // P, d], PDT, isOutput=False
    )
    # Partial outputs in bf16 (host sums the head groups in f32).
    out_d = nc.declare_dram_parameter("out", [n, d], BF16, isOutput=True)

    kc2_n = dq // P     # contraction chunks for the output projection
    on_n = d // 512     # output-column tiles

    with tile.TileContext(nc) as tc:
        with (
            # est ring: [128, 2048] bf16 tiles (sub x half x 512). Deeper
            # than the OV lag so exp drains never back-pressure.
            tc.tile_pool(name="big", bufs=12) as big,
            tc.tile_pool(name="consts", bufs=1) as consts,
            tc.tile_pool(name="weights", bufs=1) as weights,
            tc.tile_pool(name="qkt", bufs=1) as qkt,
            tc.tile_pool(name="vpool", bufs=1) as vpool,
            tc.tile_pool(name="otc", bufs=2) as otc_pool,
            tc.tile_pool(name="recip", bufs=2) as recip,
            tc.tile_pool(name="aot", bufs=2) as aot_pool,
            tc.tile_pool(name="outsb", bufs=3) as outsb,
            tc.tile_pool(name="st_ps", bufs=2, space="PSUM") as st_ps,
            tc.tile_pool(name="ot_ps", bufs=2, space="PSUM") as ot_ps,
            tc.tile_pool(name="proj_ps", bufs=2, space="PSUM") as proj_ps,
        ):
            # Weights stream on the ACT hwdge queue (q cols gate the first
            # matmul), all x halves on the SP queue, in consumption order.
            wqkv_sb = weights.tile([P, ndc, 3 * dq], PDT, tag="wqkv")
            nc.scalar.dma_start(out=wqkv_sb[:, :, 0:dq], in_=wqkv_d[:, :, 0:dq])
            nc.scalar.dma_start(
                out=wqkv_sb[:, :, dq : 2 * dq], in_=wqkv_d[:, :, dq : 2 * dq]
            )
            xbig = consts.tile([P, ndc, n], PDT, tag="xbig")
            for h2 in range(n_halves):
                for dcg in range(2):
                    # h0 fully on SP; later halves split SP / ACT so the
                    # aggregate keeps ahead of the projection stream.
                    eng = nc.sync if (h2 == 0 or dcg == 0) else nc.scalar
                    eng.dma_start(
                        out=xbig[:, dcg * ndc // 2 : (dcg + 1) * ndc // 2,
                                 h2 * nh : (h2 + 1) * nh],
                        in_=xT_d[h2, :, dcg * ndc // 2 : (dcg + 1) * ndc // 2],
                    )
            nc.scalar.dma_start(
                out=wqkv_sb[:, :, 2 * dq : 3 * dq],
                in_=wqkv_d[:, :, 2 * dq : 3 * dq],
            )
            # Late load: only needed by the output projection.
            wout_sb = weights.tile([P, kc2_n, d], PDT, tag="wout")
            nc.scalar.dma_start(out=wout_sb, in_=wout_d[:])

            def xt_slice(dc, lo, size):
                return xbig[:, dc, lo : lo + size]

            ones_bf = consts.tile([P, nt16 * hl], PDT, tag="ones_bf")
            nc.vector.memset(ones_bf, 1.0)

            # q^T / k^T projections: [128 rows = head-pair x 64 dims, tokens].
            qT = qkt.tile([P, hp_n, n], ADT, tag="qT")
            kT = qkt.tile([P, hp_n, n], ADT, tag="kT")
            for h2 in range(n_halves):
                for hp in range(hp_n):
                    pss = {}
                    for part in range(2):  # 0 = q, 1 = k
                        pss[part] = proj_ps.tile(
                            [P, 512], F32, tag="proj", name="psqk"
                        )
                    # Split the first half's chains by dim-halves so the PE
                    # starts as soon as x dc0-3 and the q columns land.
                    dc_groups = (
                        [range(0, ndc // 2), range(ndc // 2, ndc)]
                        if h2 == 0
                        else [range(ndc)]
                    )
                    for dcg in dc_groups:
                        for part in range(2):
                            off = part * dq + hp * P
                            for dc in dcg:
                                nc.tensor.matmul(
                                    pss[part],
                                    lhsT=wqkv_sb[:, dc, off : off + P],
                                    rhs=xt_slice(dc, h2 * 512, 512),
                                    start=(dc == 0),
                                    stop=(dc == ndc - 1),
                                )
                    for part in range(2):
                        dst = qT if part == 0 else kT
                        nc.vector.tensor_copy(
                            dst[:, hp, h2 * 512 : (h2 + 1) * 512], pss[part]
                        )

            # v projection in natural orientation [token, head*dh], with a
            # ones column appended per head (softmax denominator trick).
            v_sb = vpool.tile([P, nt16, hl, dh + 1], ADT, tag="v")
            nc.vector.tensor_copy(
                v_sb[:, :, :, dh : dh + 1].rearrange("p a b o -> p (a b o)"),
                ones_bf,
            )

            def emit_v(ntt):
                ps = proj_ps.tile([P, dq], F32, tag="proj", name="psv")
                for dc in range(ndc):
                    nc.tensor.matmul(
                        ps,
                        lhsT=xt_slice(dc, ntt * P, P),
                        rhs=wqkv_sb[:, dc, 2 * dq : 3 * dq],
                        start=(dc == 0),
                        stop=(dc == ndc - 1),
                    )
                nc.vector.tensor_copy(
                    v_sb[:, ntt, :, 0:dh],
                    ps.rearrange("p (h dd) -> p h dd", h=hl),
                )

            # Attention + output projection, one query group (512) at a
            # time, software-pipelined across engines:
            #   PE:     scores (row-packed head pair) -> OV (lagged ov_delay)
            #   ACT:    Exp of 9/16 of the (kcp, half) units; otc drains
            #   DVE:    fast-exp of 7/16; denominator reciprocal; tail mults
            #   GPSIMD: 1/denom broadcast + pipelined normalize multiplies
            out_ap = out_d[:]
            pending_norm = []
            pending_mults = []
            pending_outproj = []

            def emit_otcopy(qg, hp, ots, otc):
                for sub in range(2):
                    u = hp * 2 + sub
                    nc.scalar.copy(otc[:, u, :], ots[sub][0 : dh + 1, :])

            def emit_norm(qg, otc, aot, hps=(0, 1), tail=False):
                # Denominator rows sit side by side on partition 64 of otc;
                # SBUF->SBUF DMA spreads them over separate partitions so ONE
                # DVE fast-reciprocal covers them lane-parallel, then a DMA
                # brings the results back to partition 0 for the broadcasts.
                us = [hp * 2 + s for hp in hps for s in range(2)]
                nu = len(us)
                dq_eng = nc.sync
                dpn = recip.tile([nu, 512], F32, tag="dp4", name=f"dp{qg}_{us[0]}")
                dq_eng.dma_start(
                    out=dpn,
                    in_=otc[dh : dh + 1, us[0] : us[0] + nu, :].rearrange(
                        "o u t -> o (u t)"
                    ),
                )
                rrn = recip.tile([nu, 512], F32, tag="rr4", name=f"rr{qg}_{us[0]}")
                nc.vector.reciprocal_approx_fast(out=rrn, in_=dpn)
                rrow = recip.tile([1, nu, 512], F32, tag="rrow", name=f"rw{qg}_{us[0]}")
                dq_eng.dma_start(
                    out=rrow[0:1].rearrange("o u t -> o (u t)"),
                    in_=rrn,
                )
                if tail:
                    norm_mults(otc, aot, rrow, us, 0, len(us), tail=True)
                else:
                    pending_mults.append([otc, aot, rrow, us, 0])

            def norm_mults(otc, aot, rrow, us, lo, hi, tail=False):
                # bcast on gpsimd always; the multiply rides gpsimd for
                # pipelined groups (idle engine) but DVE on the tail chain.
                mul_eng = nc.vector if tail else nc.gpsimd
                for i in range(lo, hi):
                    u = us[i]
                    hp, sub = u // 2, u % 2
                    rb = recip.tile([dh, 512], F32, tag="rbcast", name="rb")
                    nc.gpsimd.partition_broadcast(rb, rrow[0:1, i, :], channels=dh)
                    mul_eng.tensor_mul(
                        out=aot[sub * dh : (sub + 1) * dh, hp, :],
                        in0=otc[0:dh, u, :],
                        in1=rb,
                    )

            def pop_mults(count):
                if not pending_mults:
                    return
                ent = pending_mults[0]
                otc_, aot_, rrow_, us_, done = ent
                hi = min(done + count, len(us_))
                if hi > done:
                    norm_mults(otc_, aot_, rrow_, us_, done, hi)
                    ent[4] = hi
                if ent[4] >= len(us_):
                    pending_mults.pop(0)

            def emit_outproj_j(qg, aot, j, pool=None, ptag="proj"):
                # One 128-token row block: kc2-major matmul order reuses
                # each stationary for both output-column tiles; drains split
                # DVE/ACT into one [128, 1024] staging tile, single store.
                ntt = qg * 4 + j
                pool = pool or proj_ps
                pss = [
                    pool.tile([P, 512], F32, tag=ptag, name="pso")
                    for _ in range(on_n)
                ]
                for kc2 in range(kc2_n):
                    for on in range(on_n):
                        nc.tensor.matmul(
                            pss[on],
                            lhsT=aot[:, kc2, j * P : (j + 1) * P],
                            rhs=wout_sb[:, kc2, on * 512 : (on + 1) * 512],
                            start=(kc2 == 0),
                            stop=(kc2 == kc2_n - 1),
                        )
                ob = outsb.tile([P, on_n * 512], BF16, tag="outsb", name="ob")
                for on in range(on_n):
                    if on % 2 == 0:
                        nc.vector.tensor_copy(ob[:, on * 512 : (on + 1) * 512], pss[on])
                    else:
                        nc.scalar.copy(ob[:, on * 512 : (on + 1) * 512], pss[on])
                eng = nc.sync if j % 2 == 0 else nc.scalar
                eng.dma_start(
                    out=out_ap[ntt * P : (ntt + 1) * P, :],
                    in_=ob,
                )

            def emit_outproj(qg, aot, pool=None, ptag="proj"):
                for j in range(4):
                    emit_outproj_j(qg, aot, j, pool=pool, ptag=ptag)

            # The OV queue carries across head-pair and query-group
            # boundaries: while the tail OVs of one block wait on their exp
            # results, the next block's score matmuls keep the PE busy.
            ov_q = []

            def do_ov(ctx, kc, est4, half):
                qg, hp, ots, otc, aot = ctx
                for sub in range(2):
                    nc.tensor.matmul(
                        ots[sub],
                        lhsT=v_sb[:, kc, hp * 2 + sub, :],
                        rhs=est4[:, sub, half, :],
                        start=(kc == 0),
                        stop=(kc == kc_n - 1),
                    )
                if kc == kc_n - 1:
                    emit_otcopy(qg, hp, ots, otc)
                    if qg == qg_n - 1:
                        emit_norm(qg, otc, aot, hps=(hp,), tail=True)
                        if hp == hp_n - 1:
                            emit_outproj(qg, aot)
                    elif hp == hp_n - 1:
                        pending_norm.append((qg, otc, aot))
                        pending_outproj.append((qg, aot, 0))

            for qg in range(qg_n):
                qs = slice(qg * 512, (qg + 1) * 512)
                aot = aot_pool.tile([P, kc2_n, 512], PDT, tag="aot", name=f"aot{qg}")
                otc = otc_pool.tile([dh + 1, 4, 512], F32, tag="otc", name=f"otc{qg}")
                for hp in range(hp_n):
                    ots = [
                        ot_ps.tile([dh + 1, 512], F32, tag="ot", name=f"ot{qg}_{hp}_{s}")
                        for s in range(2)
                    ]
                    ctx = (qg, hp, ots, otc, aot)
                    for kcp in range(kc_n // 2):
                        if qg == 0 and hp == 0:
                            emit_v(2 * kcp)
                            emit_v(2 * kcp + 1)
                        est4 = big.tile([P, 2, 2, 512], ADT, tag="big",
                                        name=f"est{qg}_{hp}_{kcp}")
                        # S^T chunks [128 keys, 512 queries] (K=64), sub0/
                        # sub1 into the two banks of one [128, 2, 512] psum
                        # tile; ONE exp op per (kcp, half) covers both subs.
                        for half in range(2):
                            kc = kcp * 2 + half
                            stp = st_ps.tile([P, 2, 512], F32, tag="st", name="stp")
                            for sub in range(2):
                                nc.tensor.matmul(
                                    stp[:, sub, :],
                                    lhsT=kT[sub * dh : (sub + 1) * dh, hp, kc * P : (kc + 1) * P],
                                    rhs=qT[sub * dh : (sub + 1) * dh, hp, qs],
                                    start=True,
                                    stop=True,
                                    tile_position=(sub * dh, 0),
                                )
                            dst = est4[:, :, half, :]
                            if dve_unit[kcp * 2 + half]:
                                # Schraudolph fast-exp on DVE: one mult-add
                                # into int16 bits, bitcast bf16.
                                nc.vector.tensor_scalar(
                                    out=dst.bitcast(I16),
                                    in0=stp,
                                    scalar1=EXP_A,
                                    scalar2=EXP_B,
                                    op0=mybir.AluOpType.mult,
                                    op1=mybir.AluOpType.add,
                                )
                            else:
                                nc.scalar.activation(
                                    out=dst,
                                    in_=stp,
                                    func=mybir.ActivationFunctionType.Exp,
                                )
                        for half in range(2):
                            ov_q.append((ctx, kcp * 2 + half, est4, half))
                        while len(ov_q) > ov_delay:
                            do_ov(*ov_q.pop(0))
                        if pending_norm and (hp == 0 and kcp >= 3 or hp == 1):
                            emit_norm(*pending_norm.pop(0))
                        if hp == 0 and kcp in (6, 7):
                            pop_mults(2)
                        elif hp == 1:
                            pop_mults(4)
                        if hp == 1 and kcp % 2 == 1 and pending_outproj:
                            pqg, paot, pj = pending_outproj[0]
                            emit_outproj_j(pqg, paot, pj)
                            if pj == 3:
                                pending_outproj.pop(0)
                            else:
                                pending_outproj[0] = (pqg, paot, pj + 1)
            for item in ov_q:
                do_ov(*item)
            pop_mults(4)
            for item in pending_norm:
                emit_norm(*item)
            pop_mults(4)
            for pqg, paot, pj in pending_outproj:
                for j in range(pj, 4):
                    emit_outproj_j(pqg, paot, j)
    nc.finalize()
    return nc


_NC_CACHE = {}


def _get_nc(mode="v3"):
    if mode not in _NC_CACHE:
        _NC_CACHE[mode] = build_attention_kernel_v4()
    return _NC_CACHE[mode]


def shard_inputs(x, gamma, w_qkv, w_out, mode="v3"):
    """FULL inputs -> list of 8 per-core input maps.

    Host-side prep (fp64): gamma+1 and the 1/sqrt(dh) attention scale are
    folded into w_qkv; the per-token rms scale rs = sqrt(d)/||x_t|| is
    folded into xT.
    """
    import ml_dtypes

    pdt = ml_dtypes.bfloat16
    d = x.shape[-1]
    dq = w_out.shape[0] // 4
    scale = DH ** -0.5
    gp1 = gamma.astype(np.float64) + 1.0
    w = w_qkv.astype(np.float64) * gp1[:, None]
    w[:, :d] *= scale  # q columns also absorb the softmax scale
    xs = x.astype(np.float64)
    rs = (d ** 0.5) / np.maximum(np.linalg.norm(xs, axis=-1), 1e-12)  # [b, n]
    xn = xs * rs[:, :, None]  # rms-normalized x (gamma fold lives in w)
    in_maps = []
    for c in range(NCORES):
        bi, g = c // 4, c % 4
        cs = slice(g * dq, (g + 1) * dq)
        wqkv_s = np.concatenate(
            [w[:, cs], w[:, d:][:, cs], w[:, 2 * d:][:, cs]], axis=1
        )
        xt = xn[bi].T.astype(pdt)  # [d, n]
        nhv = 4
        xt_tiled = np.ascontiguousarray(
            xt.reshape(d // P, P, nhv, x.shape[1] // nhv).transpose(2, 1, 0, 3)
        )
        wq = wqkv_s.astype(pdt)  # [d, 3*dq]
        wq_tiled = np.ascontiguousarray(
            wq.reshape(d // P, P, 3 * dq).transpose(1, 0, 2)
        )
        wo = w_out[cs, :].astype(pdt)  # [dq, d]
        wo_tiled = np.ascontiguousarray(
            wo.reshape(dq // P, P, d).transpose(1, 0, 2)
        )
        in_maps.append(
            {
                "xT": xt_tiled,
                "wqkv": wq_tiled,
                "wout": wo_tiled,
            }
        )
    return in_maps


def unshard_outputs(results):
    """8 partial [N, D] outputs -> full [B, N, D] (sum head groups per batch)."""
    outs = [np.asarray(r["out"], dtype=np.float32) for r in results]
    return np.stack(
        [
            outs[0] + outs[1] + outs[2] + outs[3],
            outs[4] + outs[5] + outs[6] + outs[7],
        ]
    ).astype(np.float32)


def run(x, gamma, w_qkv, w_out, mode="v3", **spmd_kwargs):
    nc = _get_nc(mode)
    in_maps = shard_inputs(x, gamma, w_qkv, w_out, mode)
    res = run_bass_kernel_spmd(nc, in_maps, list(range(NCORES)), **spmd_kwargs)
    return unshard_outputs(res.results), res


def kernel(x, gamma, w_qkv, w_out):
    out, _ = run(
        np.asarray(x), np.asarray(gamma), np.asarray(w_qkv), np.asarray(w_out)
    )
    return out


# revision 3
# speedup vs baseline: 1.4355x; 1.4355x over previous
"""Trainium2 Bass kernel for dense multi-head self-attention.

Reference computation (fp32):
    xn  = rms_norm(x) * (gamma + 1)          # F.normalize(x) * sqrt(D) * (gamma+1)
    qkv = xn @ w_qkv ; split into q, k, v    # heads H=16, dim_head 64
    out = softmax(q k^T / sqrt(64)) v
    y   = out @ w_out
Sharding (8 cores): data-parallel over batch (2), tensor-parallel over heads
(16 -> 4 groups of 4). Core c handles batch c//4, head group c%4. w_qkv is
column-sliced, w_out row-sliced per head group; each core emits a partial
[2048, 1024] output which the host sums per batch. No cross-device
communication inside the kernel.

v4 (builds on v3's 226us all-bf16 design; fp8 was measured and rejected:
elementwise fp8 noise passes straight through the random-walk attention
sums -> 4.7%+ absmax vs the 2% budget):
  - exp ops merged: ONE activation / tensor_scalar per (kcp, half) covering
    BOTH subs ([128, 2, 512] psum -> strided est write), halving ACT/DVE
    instruction count and semaphore traffic. st psum tiles are [128, 2, 512]
    (2 banks); sub0/sub1 score matmuls fill the two banks.
  - denominator reciprocals use reciprocal_approx_fast (~5x faster, 18-bit).
  - pipelined normalization multiplies move to GPSIMD (8% busy) right after
    its partition_broadcast; the last query group keeps DVE mults for the
    shortest tail chain.
  - head: weights stream on the ACT queue, all x halves on the SP queue; the
    first 512-token q/k projection chains are emitted split by dim-halves so
    the PE starts ~1.4us in and never gaps long enough to re-throttle HAM.
  - output stores merged to one [128, 1024] DMA per row block; norm DMAs on
    the SP queue; ACT engine keeps only its exp/copy work.
"""

import numpy as np

import concourse.bass as bass  # noqa: F401
import concourse.mybir as mybir
import concourse.tile as tile
from concourse import bacc
from concourse.bass_utils import run_bass_kernel_spmd

# Problem constants (hardcoded per contract; kernel.py must be self-contained).
B = 2          # batch
N = 2048       # sequence length
D = 1024       # model dim
H = 16         # total heads
DH = 64        # dim per head
HL = 4         # heads per core
DQ = HL * DH   # 256 = per-core q/k/v width
NCORES = 8

P = 128        # partitions

F32 = mybir.dt.float32
BF16 = mybir.dt.bfloat16
I16 = mybir.dt.int16

# Schraudolph fast-exp constants in bf16-exponent space:
#   bits = s * (2^7 / ln 2) + (127*2^7 - c);  bitcast<bf16>(bits) ~ exp(s)
# c = 2^7 * 0.043 balances the (1+f)/2^f linear-interp error to +-3%.
EXP_A = 128.0 / np.log(2.0)
EXP_B = 16256.0 - 5.513


def build_attention_kernel_v4(n=N, d=D, hl=HL, dh=DH):
    """Build the single-core SPMD Bass program (v4, all-bf16)."""
    PDT = BF16
    ADT = BF16
    dq = hl * dh
    ndc = d // P        # dim chunks of 128
    nt4 = n // 512      # token tiles of 512
    nt16 = n // P       # token tiles of 128
    kc_n = n // P       # key chunks of 128
    qg_n = n // 512     # query groups of 512
    hp_n = hl // 2      # head pairs
    ov_delay = 6

    # Routing of the 16 (kcp, half) exp units per (qg, hp) to DVE: strict
    # half-alternation (half0 -> ACT, half1 -> DVE) except u=11 -> ACT, where
    # the per-qg DVE reciprocal is scheduled so it never delays a fast-exp.
    dve_unit = [(u % 2 == 1) and u != 11 for u in range(16)]

    nc = bacc.Bacc()
    # xT arrives already rms-normalized (host folds rs[t] = sqrt(d)/||x_t||
    # into the columns). All inputs are HOST-PRE-TILED to the on-chip layout
    # so every DMA reads long contiguous runs.
    n_halves = 4 if n >= 2048 else (2 if n >= 1024 else 1)
    nh = n // n_halves
    xT_d = nc.declare_dram_parameter(
        "xT", [n_halves, P, d // P, nh], PDT, isOutput=False
    )
    wqkv_d = nc.declare_dram_parameter(
        "wqkv", [P, d // P, 3 * dq], PDT, isOutput=False
    )
    wout_d = nc.declare_dram_parameter(
        "wout", [P, dq // P, d], PDT, isOutput=False
    )
    # Partial outputs in bf16 (host sums the head groups in f32).
    out_d = nc.declare_dram_parameter("out", [n, d], BF16, isOutput=True)

    kc2_n = dq // P     # contraction chunks for the output projection
    on_n = d // 512     # output-column tiles

    with tile.TileContext(nc) as tc:
        with (
            # est ring: [128, 2048] bf16 tiles (sub x half x 512). Deeper
            # than the OV lag so exp drains never back-pressure.
            tc.tile_pool(name="big", bufs=12) as big,
            tc.tile_pool(name="consts", bufs=1) as consts,
            tc.tile_pool(name="weights", bufs=1) as weights,
            tc.tile_pool(name="qkt", bufs=1) as qkt,
            tc.tile_pool(name="vpool", bufs=1) as vpool,
            tc.tile_pool(name="otc", bufs=2) as otc_pool,
            tc.tile_pool(name="recip", bufs=2) as recip,
            tc.tile_pool(name="aot", bufs=2) as aot_pool,
            tc.tile_pool(name="outsb", bufs=3) as outsb,
            tc.tile_pool(name="st_ps", bufs=2, space="PSUM") as st_ps,
            tc.tile_pool(name="ot_ps", bufs=2, space="PSUM") as ot_ps,
            tc.tile_pool(name="proj_ps", bufs=2, space="PSUM") as proj_ps,
        ):
            # Weights stream on the ACT hwdge queue (q cols gate the first
            # matmul), all x halves on the SP queue, in consumption order.
            wqkv_sb = weights.tile([P, ndc, 3 * dq], PDT, tag="wqkv")
            nc.scalar.dma_start(out=wqkv_sb[:, :, 0:dq], in_=wqkv_d[:, :, 0:dq])
            nc.scalar.dma_start(
                out=wqkv_sb[:, :, dq : 2 * dq], in_=wqkv_d[:, :, dq : 2 * dq]
            )
            xbig = consts.tile([P, ndc, n], PDT, tag="xbig")
            for h2 in range(n_halves):
                for dcg in range(2):
                    # h0 fully on SP; later halves split SP / ACT so the
                    # aggregate keeps ahead of the projection stream.
                    eng = nc.sync if (h2 == 0 or dcg == 0) else nc.scalar
                    eng.dma_start(
                        out=xbig[:, dcg * ndc // 2 : (dcg + 1) * ndc // 2,
                                 h2 * nh : (h2 + 1) * nh],
                        in_=xT_d[h2, :, dcg * ndc // 2 : (dcg + 1) * ndc // 2],
                    )
            nc.scalar.dma_start(
                out=wqkv_sb[:, :, 2 * dq : 3 * dq],
                in_=wqkv_d[:, :, 2 * dq : 3 * dq],
            )
            # Late load: only needed by the output projection.
            wout_sb = weights.tile([P, kc2_n, d], PDT, tag="wout")
            nc.scalar.dma_start(out=wout_sb, in_=wout_d[:])

            def xt_slice(dc, lo, size):
                return xbig[:, dc, lo : lo + size]

            ones_bf = consts.tile([P, nt16 * hl], PDT, tag="ones_bf")
            nc.vector.memset(ones_bf, 1.0)

            # q^T / k^T projections: [128 rows = head-pair x 64 dims, tokens].
            qT = qkt.tile([P, hp_n, n], ADT, tag="qT")
            kT = qkt.tile([P, hp_n, n], ADT, tag="kT")
            for h2 in range(n_halves):
                for hp in range(hp_n):
                    pss = {}
                    for part in range(2):  # 0 = q, 1 = k
                        pss[part] = proj_ps.tile(
                            [P, 512], F32, tag="proj", name="psqk"
                        )
                    # Split the first half's chains by dim-halves so the PE
                    # starts as soon as x dc0-3 and the q columns land.
                    dc_groups = (
                        [range(0, ndc // 2), range(ndc // 2, ndc)]
                        if h2 == 0
                        else [range(ndc)]
                    )
                    for dcg in dc_groups:
                        for part in range(2):
                            off = part * dq + hp * P
                            for dc in dcg:
                                nc.tensor.matmul(
                                    pss[part],
                                    lhsT=wqkv_sb[:, dc, off : off + P],
                                    rhs=xt_slice(dc, h2 * 512, 512),
                                    start=(dc == 0),
                                    stop=(dc == ndc - 1),
                                )
                    for part in range(2):
                        dst = qT if part == 0 else kT
                        nc.vector.tensor_copy(
                            dst[:, hp, h2 * 512 : (h2 + 1) * 512], pss[part]
                        )

            # v projection in natural orientation [token, head*dh], with a
            # ones column appended per head (softmax denominator trick).
            v_sb = vpool.tile([P, nt16, hl, dh + 1], ADT, tag="v")
            nc.vector.tensor_copy(
                v_sb[:, :, :, dh : dh + 1].rearrange("p a b o -> p (a b o)"),
                ones_bf,
            )

            def emit_v(ntt):
                ps = proj_ps.tile([P, dq], F32, tag="proj", name="psv")
                for dc in range(ndc):
                    nc.tensor.matmul(
                        ps,
                        lhsT=xt_slice(dc, ntt * P, P),
                        rhs=wqkv_sb[:, dc, 2 * dq : 3 * dq],
                        start=(dc == 0),
                        stop=(dc == ndc - 1),
                    )
                nc.vector.tensor_copy(
                    v_sb[:, ntt, :, 0:dh],
                    ps.rearrange("p (h dd) -> p h dd", h=hl),
                )

            # Attention + output projection, one query group (512) at a
            # time, software-pipelined across engines:
            #   PE:     scores (row-packed head pair) -> OV (lagged ov_delay)
            #   ACT:    Exp of 9/16 of the (kcp, half) units; otc drains
            #   DVE:    fast-exp of 7/16; denominator reciprocal; tail mults
            #   GPSIMD: 1/denom broadcast + pipelined normalize multiplies
            out_ap = out_d[:]
            pending_norm = []
            pending_mults = []
            pending_outproj = []

            def emit_otcopy(qg, hp, ots, otc):
                for sub in range(2):
                    u = hp * 2 + sub
                    nc.scalar.copy(otc[:, u, :], ots[sub][0 : dh + 1, :])

            def emit_norm(qg, otc, aot, hps=(0, 1), tail=False):
                # Denominator rows sit side by side on partition 64 of otc;
                # SBUF->SBUF DMA spreads them over separate partitions so ONE
                # DVE fast-reciprocal covers them lane-parallel, then a DMA
                # brings the results back to partition 0 for the broadcasts.
                us = [hp * 2 + s for hp in hps for s in range(2)]
                nu = len(us)
                dq_eng = nc.sync
                dpn = recip.tile([nu, 512], F32, tag="dp4", name=f"dp{qg}_{us[0]}")
                dq_eng.dma_start(
                    out=dpn,
                    in_=otc[dh : dh + 1, us[0] : us[0] + nu, :].rearrange(
                        "o u t -> o (u t)"
                    ),
                )
                rrn = recip.tile([nu, 512], F32, tag="rr4", name=f"rr{qg}_{us[0]}")
                nc.vector.reciprocal_approx_fast(out=rrn, in_=dpn)
                rrow = recip.tile([1, nu, 512], F32, tag="rrow", name=f"rw{qg}_{us[0]}")
                dq_eng.dma_start(
                    out=rrow[0:1].rearrange("o u t -> o (u t)"),
                    in_=rrn,
                )
                if tail:
                    norm_mults(otc, aot, rrow, us, 0, len(us), tail=True)
                else:
                    pending_mults.append([otc, aot, rrow, us, 0])

            def norm_mults(otc, aot, rrow, us, lo, hi, tail=False):
                # bcast on gpsimd, multiply on DVE. Do NOT move the multiply
                # to gpsimd: TENSOR_TENSOR lives in a different gpsimd ucode
                # library than partition_broadcast, and each switch costs a
                # 5-7us UNLOAD_LIB/LOAD_LIB pair (measured) that stalls the
                # whole normalization chain.
                mul_eng = nc.vector
                for i in range(lo, hi):
                    u = us[i]
                    hp, sub = u // 2, u % 2
                    rb = recip.tile([dh, 512], F32, tag="rbcast", name="rb")
                    nc.gpsimd.partition_broadcast(rb, rrow[0:1, i, :], channels=dh)
                    mul_eng.tensor_mul(
                        out=aot[sub * dh : (sub + 1) * dh, hp, :],
                        in0=otc[0:dh, u, :],
                        in1=rb,
                    )

            def pop_mults(count):
                if not pending_mults:
                    return
                ent = pending_mults[0]
                otc_, aot_, rrow_, us_, done = ent
                hi = min(done + count, len(us_))
                if hi > done:
                    norm_mults(otc_, aot_, rrow_, us_, done, hi)
                    ent[4] = hi
                if ent[4] >= len(us_):
                    pending_mults.pop(0)

            def emit_outproj_j(qg, aot, j, pool=None, ptag="proj"):
                # One 128-token row block: kc2-major matmul order reuses
                # each stationary for both output-column tiles; drains split
                # DVE/ACT into one [128, 1024] staging tile, single store.
                ntt = qg * 4 + j
                pool = pool or proj_ps
                pss = [
                    pool.tile([P, 512], F32, tag=ptag, name="pso")
                    for _ in range(on_n)
                ]
                for kc2 in range(kc2_n):
                    for on in range(on_n):
                        nc.tensor.matmul(
                            pss[on],
                            lhsT=aot[:, kc2, j * P : (j + 1) * P],
                            rhs=wout_sb[:, kc2, on * 512 : (on + 1) * 512],
                            start=(kc2 == 0),
                            stop=(kc2 == kc2_n - 1),
                        )
                ob = outsb.tile([P, on_n * 512], BF16, tag="outsb", name="ob")
                for on in range(on_n):
                    if on % 2 == 0:
                        nc.vector.tensor_copy(ob[:, on * 512 : (on + 1) * 512], pss[on])
                    else:
                        nc.scalar.copy(ob[:, on * 512 : (on + 1) * 512], pss[on])
                eng = nc.sync if j % 2 == 0 else nc.scalar
                eng.dma_start(
                    out=out_ap[ntt * P : (ntt + 1) * P, :],
                    in_=ob,
                )

            def emit_outproj(qg, aot, pool=None, ptag="proj"):
                for j in range(4):
                    emit_outproj_j(qg, aot, j, pool=pool, ptag=ptag)

            # The OV queue carries across head-pair and query-group
            # boundaries: while the tail OVs of one block wait on their exp
            # results, the next block's score matmuls keep the PE busy.
            ov_q = []

            def do_ov(ctx, kc, est4, half):
                qg, hp, ots, otc, aot = ctx
                for sub in range(2):
                    nc.tensor.matmul(
                        ots[sub],
                        lhsT=v_sb[:, kc, hp * 2 + sub, :],
                        rhs=est4[:, sub, half, :],
                        start=(kc == 0),
                        stop=(kc == kc_n - 1),
                    )
                if kc == kc_n - 1:
                    emit_otcopy(qg, hp, ots, otc)
                    if qg == qg_n - 1:
                        emit_norm(qg, otc, aot, hps=(hp,), tail=True)
                        if hp == hp_n - 1:
                            emit_outproj(qg, aot)
                    elif hp == hp_n - 1:
                        pending_norm.append((qg, otc, aot))
                        pending_outproj.append((qg, aot, 0))

            for qg in range(qg_n):
                qs = slice(qg * 512, (qg + 1) * 512)
                aot = aot_pool.tile([P, kc2_n, 512], PDT, tag="aot", name=f"aot{qg}")
                otc = otc_pool.tile([dh + 1, 4, 512], F32, tag="otc", name=f"otc{qg}")
                for hp in range(hp_n):
                    ots = [
                        ot_ps.tile([dh + 1, 512], F32, tag="ot", name=f"ot{qg}_{hp}_{s}")
                        for s in range(2)
                    ]
                    ctx = (qg, hp, ots, otc, aot)
                    for kcp in range(kc_n // 2):
                        if qg == 0 and hp == 0:
                            emit_v(2 * kcp)
                            emit_v(2 * kcp + 1)
                        est4 = big.tile([P, 2, 2, 512], ADT, tag="big",
                                        name=f"est{qg}_{hp}_{kcp}")
                        # S^T chunks [128 keys, 512 queries] (K=64), sub0/
                        # sub1 into the two banks of one [128, 2, 512] psum
                        # tile; ONE exp op per (kcp, half) covers both subs.
                        for half in range(2):
                            kc = kcp * 2 + half
                            stp = st_ps.tile([P, 2, 512], F32, tag="st", name="stp")
                            for sub in range(2):
                                nc.tensor.matmul(
                                    stp[:, sub, :],
                                    lhsT=kT[sub * dh : (sub + 1) * dh, hp, kc * P : (kc + 1) * P],
                                    rhs=qT[sub * dh : (sub + 1) * dh, hp, qs],
                                    start=True,
                                    stop=True,
                                    tile_position=(sub * dh, 0),
                                )
                            dst = est4[:, :, half, :]
                            if dve_unit[kcp * 2 + half]:
                                # Schraudolph fast-exp on DVE: one mult-add
                                # into int16 bits, bitcast bf16.
                                nc.vector.tensor_scalar(
                                    out=dst.bitcast(I16),
                                    in0=stp,
                                    scalar1=EXP_A,
                                    scalar2=EXP_B,
                                    op0=mybir.AluOpType.mult,
                                    op1=mybir.AluOpType.add,
                                )
                            else:
                                nc.scalar.activation(
                                    out=dst,
                                    in_=stp,
                                    func=mybir.ActivationFunctionType.Exp,
                                )
                        for half in range(2):
                            ov_q.append((ctx, kcp * 2 + half, est4, half))
                        while len(ov_q) > ov_delay:
                            do_ov(*ov_q.pop(0))
                        if pending_norm and (hp == 0 and kcp >= 3 or hp == 1):
                            emit_norm(*pending_norm.pop(0))
                        if hp == 0 and kcp in (6, 7):
                            pop_mults(2)
                        elif hp == 1:
                            pop_mults(4)
                        if hp == 1 and kcp % 2 == 1 and pending_outproj:
                            pqg, paot, pj = pending_outproj[0]
                            emit_outproj_j(pqg, paot, pj)
                            if pj == 3:
                                pending_outproj.pop(0)
                            else:
                                pending_outproj[0] = (pqg, paot, pj + 1)
            for item in ov_q:
                do_ov(*item)
            pop_mults(4)
            for item in pending_norm:
                emit_norm(*item)
            pop_mults(4)
            for pqg, paot, pj in pending_outproj:
                for j in range(pj, 4):
                    emit_outproj_j(pqg, paot, j)
    nc.finalize()
    return nc


_NC_CACHE = {}


def _get_nc(mode="v3"):
    if mode not in _NC_CACHE:
        _NC_CACHE[mode] = build_attention_kernel_v4()
    return _NC_CACHE[mode]


def shard_inputs(x, gamma, w_qkv, w_out, mode="v3"):
    """FULL inputs -> list of 8 per-core input maps.

    Host-side prep (fp64): gamma+1 and the 1/sqrt(dh) attention scale are
    folded into w_qkv; the per-token rms scale rs = sqrt(d)/||x_t|| is
    folded into xT.
    """
    import ml_dtypes

    pdt = ml_dtypes.bfloat16
    d = x.shape[-1]
    dq = w_out.shape[0] // 4
    scale = DH ** -0.5
    gp1 = gamma.astype(np.float64) + 1.0
    w = w_qkv.astype(np.float64) * gp1[:, None]
    w[:, :d] *= scale  # q columns also absorb the softmax scale
    xs = x.astype(np.float64)
    rs = (d ** 0.5) / np.maximum(np.linalg.norm(xs, axis=-1), 1e-12)  # [b, n]
    xn = xs * rs[:, :, None]  # rms-normalized x (gamma fold lives in w)
    in_maps = []
    for c in range(NCORES):
        bi, g = c // 4, c % 4
        cs = slice(g * dq, (g + 1) * dq)
        wqkv_s = np.concatenate(
            [w[:, cs], w[:, d:][:, cs], w[:, 2 * d:][:, cs]], axis=1
        )
        xt = xn[bi].T.astype(pdt)  # [d, n]
        nhv = 4
        xt_tiled = np.ascontiguousarray(
            xt.reshape(d // P, P, nhv, x.shape[1] // nhv).transpose(2, 1, 0, 3)
        )
        wq = wqkv_s.astype(pdt)  # [d, 3*dq]
        wq_tiled = np.ascontiguousarray(
            wq.reshape(d // P, P, 3 * dq).transpose(1, 0, 2)
        )
        wo = w_out[cs, :].astype(pdt)  # [dq, d]
        wo_tiled = np.ascontiguousarray(
            wo.reshape(dq // P, P, d).transpose(1, 0, 2)
        )
        in_maps.append(
            {
                "xT": xt_tiled,
                "wqkv": wq_tiled,
                "wout": wo_tiled,
            }
        )
    return in_maps


def unshard_outputs(results):
    """8 partial [N, D] outputs -> full [B, N, D] (sum head groups per batch)."""
    outs = [np.asarray(r["out"], dtype=np.float32) for r in results]
    return np.stack(
        [
            outs[0] + outs[1] + outs[2] + outs[3],
            outs[4] + outs[5] + outs[6] + outs[7],
        ]
    ).astype(np.float32)


def run(x, gamma, w_qkv, w_out, mode="v3", **spmd_kwargs):
    nc = _get_nc(mode)
    in_maps = shard_inputs(x, gamma, w_qkv, w_out, mode)
    res = run_bass_kernel_spmd(nc, in_maps, list(range(NCORES)), **spmd_kwargs)
    return unshard_outputs(res.results), res


def kernel(x, gamma, w_qkv, w_out):
    out, _ = run(
        np.asarray(x), np.asarray(gamma), np.asarray(w_qkv), np.asarray(w_out)
    )
    return out


# revision 15
# speedup vs baseline: 1.4387x; 1.0022x over previous
"""Trainium2 Bass kernel for dense multi-head self-attention.

Reference computation (fp32):
    xn  = rms_norm(x) * (gamma + 1)          # F.normalize(x) * sqrt(D) * (gamma+1)
    qkv = xn @ w_qkv ; split into q, k, v    # heads H=16, dim_head 64
    out = softmax(q k^T / sqrt(64)) v
    y   = out @ w_out
Sharding (8 cores): data-parallel over batch (2), tensor-parallel over heads
(16 -> 4 groups of 4). Core c handles batch c//4, head group c%4. w_qkv is
column-sliced, w_out row-sliced per head group; each core emits a partial
[2048, 1024] output which the host sums per batch. No cross-device
communication inside the kernel.

v4 (builds on v3's 226us all-bf16 design; fp8 was measured and rejected:
elementwise fp8 noise passes straight through the random-walk attention
sums -> 4.7%+ absmax vs the 2% budget):
  - exp ops merged: ONE activation / tensor_scalar per (kcp, half) covering
    BOTH subs ([128, 2, 512] psum -> strided est write), halving ACT/DVE
    instruction count and semaphore traffic. st psum tiles are [128, 2, 512]
    (2 banks); sub0/sub1 score matmuls fill the two banks.
  - denominator reciprocals use reciprocal_approx_fast (~5x faster, 18-bit).
  - pipelined normalization multiplies move to GPSIMD (8% busy) right after
    its partition_broadcast; the last query group keeps DVE mults for the
    shortest tail chain.
  - head: weights stream on the ACT queue, all x halves on the SP queue; the
    first 512-token q/k projection chains are emitted split by dim-halves so
    the PE starts ~1.4us in and never gaps long enough to re-throttle HAM.
  - output stores merged to one [128, 1024] DMA per row block; norm DMAs on
    the SP queue; ACT engine keeps only its exp/copy work.
"""

import numpy as np

import concourse.bass as bass  # noqa: F401
import concourse.mybir as mybir
import concourse.tile as tile
from concourse import bacc
from concourse.bass_utils import run_bass_kernel_spmd

# Problem constants (hardcoded per contract; kernel.py must be self-contained).
B = 2          # batch
N = 2048       # sequence length
D = 1024       # model dim
H = 16         # total heads
DH = 64        # dim per head
HL = 4         # heads per core
DQ = HL * DH   # 256 = per-core q/k/v width
NCORES = 8

P = 128        # partitions

F32 = mybir.dt.float32
BF16 = mybir.dt.bfloat16
I16 = mybir.dt.int16

# Schraudolph fast-exp constants in bf16-exponent space:
#   bits = s * (2^7 / ln 2) + (127*2^7 - c);  bitcast<bf16>(bits) ~ exp(s)
# c = 2^7 * 0.043 balances the (1+f)/2^f linear-interp error to +-3%.
EXP_A = 128.0 / np.log(2.0)
EXP_B = 16256.0 - 5.513


def build_attention_kernel_v4(n=N, d=D, hl=HL, dh=DH):
    """Build the single-core SPMD Bass program (v4, all-bf16)."""
    PDT = BF16
    ADT = BF16
    dq = hl * dh
    ndc = d // P        # dim chunks of 128
    nt4 = n // 512      # token tiles of 512
    nt16 = n // P       # token tiles of 128
    kc_n = n // P       # key chunks of 128
    qg_n = n // 512     # query groups of 512
    hp_n = hl // 2      # head pairs
    ov_delay = 6

    # Routing of the 16 (kcp, half) exp units per (qg, hp) to DVE: strict
    # half-alternation (half0 -> ACT, half1 -> DVE) except u=11 -> ACT, where
    # the per-qg DVE reciprocal is scheduled so it never delays a fast-exp.
    dve_unit = [(u % 2 == 1) and u != 11 for u in range(16)]

    nc = bacc.Bacc()
    # xT arrives already rms-normalized (host folds rs[t] = sqrt(d)/||x_t||
    # into the columns). All inputs are HOST-PRE-TILED to the on-chip layout
    # so every DMA reads long contiguous runs.
    n_halves = 4 if n >= 2048 else (2 if n >= 1024 else 1)
    nh = n // n_halves
    xT_d = nc.declare_dram_parameter(
        "xT", [n_halves, P, d // P, nh], PDT, isOutput=False
    )
    wqkv_d = nc.declare_dram_parameter(
        "wqkv", [P, d // P, 3 * dq], PDT, isOutput=False
    )
    wout_d = nc.declare_dram_parameter(
        "wout", [P, dq // P, d], PDT, isOutput=False
    )
    # Partial outputs in bf16 (host sums the head groups in f32).
    out_d = nc.declare_dram_parameter("out", [n, d], BF16, isOutput=True)

    kc2_n = dq // P     # contraction chunks for the output projection
    on_n = d // 512     # output-column tiles

    with tile.TileContext(nc) as tc:
        with (
            # est ring: [128, 2048] bf16 tiles (sub x half x 512). Deeper
            # than the OV lag so exp drains never back-pressure.
            tc.tile_pool(name="big", bufs=12) as big,
            tc.tile_pool(name="consts", bufs=1) as consts,
            tc.tile_pool(name="weights", bufs=1) as weights,
            tc.tile_pool(name="qkt", bufs=1) as qkt,
            tc.tile_pool(name="vpool", bufs=1) as vpool,
            tc.tile_pool(name="otc", bufs=2) as otc_pool,
            tc.tile_pool(name="recip", bufs=2) as recip,
            tc.tile_pool(name="aot", bufs=2) as aot_pool,
            tc.tile_pool(name="outsb", bufs=3) as outsb,
            tc.tile_pool(name="st_ps", bufs=2, space="PSUM") as st_ps,
            tc.tile_pool(name="ot_ps", bufs=2, space="PSUM") as ot_ps,
            tc.tile_pool(name="proj_ps", bufs=2, space="PSUM") as proj_ps,
        ):
            # Measured: each hwdge queue sustains only ~70-85 GB/s, so the
            # head is DMA-latency-bound. Spread the critical first loads over
            # FOUR queues (SP, ACT, DVE, gpsimd-swdge) in consumption order:
            # the first projection chain (hp0, q, dc0-3) needs only the hp0
            # column half of the q weights plus x dc0-3 of the first 512
            # tokens, each split into 0.25 MB chunks on separate queues.
            wqkv_sb = weights.tile([P, ndc, 3 * dq], PDT, tag="wqkv")
            xbig = consts.tile([P, ndc, n], PDT, tag="xbig")

            def load_x(h2, dclo, dchi, eng):
                eng.dma_start(
                    out=xbig[:, dclo:dchi, h2 * nh : (h2 + 1) * nh],
                    in_=xT_d[h2, :, dclo:dchi],
                )

            def load_w(clo, chi, eng):
                eng.dma_start(out=wqkv_sb[:, :, clo:chi], in_=wqkv_d[:, :, clo:chi])

            load_w(0, P, nc.scalar)             # q cols, hp0 half
            load_x(0, 0, 2, nc.sync)
            load_x(0, 2, 4, nc.sync)
            load_x(0, 4, 6, nc.gpsimd)
            load_x(0, 6, 8, nc.gpsimd)
            load_w(dq, dq + P, nc.scalar)       # k cols, hp0 half
            load_w(P, dq, nc.gpsimd)            # q cols, hp1 half
            load_w(dq + P, 2 * dq, nc.gpsimd)   # k cols, hp1 half
            load_x(1, 0, 2, nc.sync)
            load_x(1, 2, 4, nc.sync)
            load_x(1, 4, 6, nc.scalar)
            load_x(1, 6, 8, nc.scalar)
            load_x(2, 0, 2, nc.sync)
            load_x(2, 2, 4, nc.sync)
            load_x(2, 4, 6, nc.scalar)
            load_x(2, 6, 8, nc.scalar)
            load_x(3, 0, 2, nc.sync)
            load_x(3, 2, 4, nc.gpsimd)
            load_x(3, 4, 6, nc.scalar)
            load_x(3, 6, 8, nc.scalar)
            load_w(2 * dq, 3 * dq, nc.scalar)   # v cols
            # Late load: only needed by the output projection.
            wout_sb = weights.tile([P, kc2_n, d], PDT, tag="wout")
            nc.scalar.dma_start(out=wout_sb, in_=wout_d[:])

            def xt_slice(dc, lo, size):
                return xbig[:, dc, lo : lo + size]

            ones_bf = consts.tile([P, nt16 * hl], PDT, tag="ones_bf")
            nc.vector.memset(ones_bf, 1.0)

            # HAM warm-up: the PE clock gate defaults to 4/8 (1.2 GHz) and
            # only opens after ~3.4us of sustained matmul activity. The head
            # is DMA-bound, so burn the wait on dummy matmuls over ones_bf;
            # the first real matmuls then run at 2.4 GHz.
            warm_ps = st_ps.tile([P, 2, 512], F32, tag="st", name="warm")
            for i in range(40):
                nc.tensor.matmul(
                    warm_ps[0:64, 0, 0:64],
                    lhsT=ones_bf[:, 0:64],
                    rhs=ones_bf[:, 0:64],
                    start=True,
                    stop=True,
                )

            # q^T / k^T projections: [128 rows = head-pair x 64 dims, tokens].
            qT = qkt.tile([P, hp_n, n], ADT, tag="qT")
            kT = qkt.tile([P, hp_n, n], ADT, tag="kT")
            for h2 in range(n_halves):
                for hp in range(hp_n):
                    pss = {}
                    for part in range(2):  # 0 = q, 1 = k
                        pss[part] = proj_ps.tile(
                            [P, 512], F32, tag="proj", name="psqk"
                        )
                    # Split the first half's chains into fine (part, dc)
                    # phases ordered by DMA arrival so the PE starts as soon
                    # as x dc0-1 and the hp0 q columns land.
                    if h2 == 0:
                        phases = [(0, 0, 2), (0, 2, 4), (1, 0, 2), (1, 2, 4),
                                  (0, 4, 6), (0, 6, 8), (1, 4, 6), (1, 6, 8)]
                    else:
                        phases = [(0, 0, ndc), (1, 0, ndc)]
                    for part, dclo, dchi in phases:
                        off = part * dq + hp * P
                        for dc in range(dclo, dchi):
                            nc.tensor.matmul(
                                pss[part],
                                lhsT=wqkv_sb[:, dc, off : off + P],
                                rhs=xt_slice(dc, h2 * 512, 512),
                                start=(dc == 0),
                                stop=(dc == ndc - 1),
                            )
                    for part in range(2):
                        dst = qT if part == 0 else kT
                        nc.vector.tensor_copy(
                            dst[:, hp, h2 * 512 : (h2 + 1) * 512], pss[part]
                        )

            # v projection in natural orientation [token, head*dh], with a
            # ones column appended per head (softmax denominator trick).
            v_sb = vpool.tile([P, nt16, hl, dh + 1], ADT, tag="v")
            nc.vector.tensor_copy(
                v_sb[:, :, :, dh : dh + 1].rearrange("p a b o -> p (a b o)"),
                ones_bf,
            )

            def emit_v(ntt):
                ps = proj_ps.tile([P, dq], F32, tag="proj", name="psv")
                for dc in range(ndc):
                    nc.tensor.matmul(
                        ps,
                        lhsT=xt_slice(dc, ntt * P, P),
                        rhs=wqkv_sb[:, dc, 2 * dq : 3 * dq],
                        start=(dc == 0),
                        stop=(dc == ndc - 1),
                    )
                nc.vector.tensor_copy(
                    v_sb[:, ntt, :, 0:dh],
                    ps.rearrange("p (h dd) -> p h dd", h=hl),
                )

            # Attention + output projection, one query group (512) at a
            # time, software-pipelined across engines:
            #   PE:     scores (row-packed head pair) -> OV (lagged ov_delay)
            #   ACT:    Exp of 9/16 of the (kcp, half) units; otc drains
            #   DVE:    fast-exp of 7/16; denominator reciprocal; tail mults
            #   GPSIMD: 1/denom broadcast + pipelined normalize multiplies
            out_ap = out_d[:]
            pending_norm = []
            pending_mults = []
            pending_outproj = []

            def emit_otcopy(qg, hp, ots, otc):
                for sub in range(2):
                    u = hp * 2 + sub
                    nc.scalar.copy(otc[:, u, :], ots[sub][0 : dh + 1, :])

            def emit_norm(qg, otc, aot, hps=(0, 1), tail=False):
                us = [hp * 2 + s for hp in hps for s in range(2)]
                nu = len(us)
                # Denominator rows sit side by side on partition 64 of otc;
                # SBUF->SBUF DMA spreads them over separate partitions so
                # ONE lane-parallel DVE fast-reciprocal covers all units,
                # then a DMA brings the results back to partition 0 for the
                # broadcasts. NOTE: do not "optimize" the DMA hops away —
                # DVE ops (incl. custom ones) and partition_broadcast cannot
                # move data across partitions on HW (lane-locked; measured
                # as garbage reads of uninitialized SBUF), so partition 64 ->
                # partition 0 requires a DMA.
                dq_eng = nc.sync
                dpn = recip.tile([nu, 512], F32, tag="dp4", name=f"dp{qg}_{us[0]}")
                dq_eng.dma_start(
                    out=dpn,
                    in_=otc[dh : dh + 1, us[0] : us[0] + nu, :].rearrange(
                        "o u t -> o (u t)"
                    ),
                )
                rrn = recip.tile([nu, 512], F32, tag="rr4", name=f"rr{qg}_{us[0]}")
                nc.vector.reciprocal_approx_fast(out=rrn, in_=dpn)
                rrow = recip.tile([1, nu, 512], F32, tag="rrow", name=f"rw{qg}_{us[0]}")
                dq_eng.dma_start(
                    out=rrow[0:1].rearrange("o u t -> o (u t)"),
                    in_=rrn,
                )
                if tail:
                    norm_mults(otc, aot, rrow, us, 0, len(us), tail=True)
                else:
                    pending_mults.append([otc, aot, rrow, us, 0])

            def norm_mults(otc, aot, rrow, us, lo, hi, tail=False):
                # bcast on gpsimd, multiply on DVE. Do NOT move the multiply
                # to gpsimd: TENSOR_TENSOR lives in a different gpsimd ucode
                # library than partition_broadcast, and each switch costs a
                # 5-7us UNLOAD_LIB/LOAD_LIB pair (measured) that stalls the
                # whole normalization chain.
                mul_eng = nc.vector
                for i in range(lo, hi):
                    u = us[i]
                    hp, sub = u // 2, u % 2
                    rb = recip.tile([dh, 512], F32, tag="rbcast", name="rb")
                    nc.gpsimd.partition_broadcast(rb, rrow[0:1, i, :], channels=dh)
                    mul_eng.tensor_mul(
                        out=aot[sub * dh : (sub + 1) * dh, hp, :],
                        in0=otc[0:dh, u, :],
                        in1=rb,
                    )

            def pop_mults(count):
                if not pending_mults:
                    return
                ent = pending_mults[0]
                otc_, aot_, rrow_, us_, done = ent
                hi = min(done + count, len(us_))
                if hi > done:
                    norm_mults(otc_, aot_, rrow_, us_, done, hi)
                    ent[4] = hi
                if ent[4] >= len(us_):
                    pending_mults.pop(0)

            def emit_outproj_j(qg, aot, j, pool=None, ptag="proj"):
                # One 128-token row block: kc2-major matmul order reuses
                # each stationary for both output-column tiles; drains split
                # DVE/ACT into one [128, 1024] staging tile, single store.
                ntt = qg * 4 + j
                pool = pool or proj_ps
                pss = [
                    pool.tile([P, 512], F32, tag=ptag, name="pso")
                    for _ in range(on_n)
                ]
                for kc2 in range(kc2_n):
                    for on in range(on_n):
                        nc.tensor.matmul(
                            pss[on],
                            lhsT=aot[:, kc2, j * P : (j + 1) * P],
                            rhs=wout_sb[:, kc2, on * 512 : (on + 1) * 512],
                            start=(kc2 == 0),
                            stop=(kc2 == kc2_n - 1),
                        )
                ob = outsb.tile([P, on_n * 512], BF16, tag="outsb", name="ob")
                for on in range(on_n):
                    if on % 2 == 0:
                        nc.vector.tensor_copy(ob[:, on * 512 : (on + 1) * 512], pss[on])
                    else:
                        nc.scalar.copy(ob[:, on * 512 : (on + 1) * 512], pss[on])
                if qg == qg_n - 1:
                    # End-of-kernel stores are latency-critical (~70 GB/s
                    # per queue): split each row block across both queues.
                    for on in range(on_n):
                        eng = nc.sync if (j + on) % 2 == 0 else nc.scalar
                        eng.dma_start(
                            out=out_ap[ntt * P : (ntt + 1) * P,
                                       on * 512 : (on + 1) * 512],
                            in_=ob[:, on * 512 : (on + 1) * 512],
                        )
                else:
                    eng = nc.sync if j % 2 == 0 else nc.scalar
                    eng.dma_start(
                        out=out_ap[ntt * P : (ntt + 1) * P, :],
                        in_=ob,
                    )

            def emit_outproj(qg, aot, pool=None, ptag="proj"):
                for j in range(4):
                    emit_outproj_j(qg, aot, j, pool=pool, ptag=ptag)

            # The OV queue carries across head-pair and query-group
            # boundaries: while the tail OVs of one block wait on their exp
            # results, the next block's score matmuls keep the PE busy.
            ov_q = []

            def do_ov(ctx, kc, est4, half):
                qg, hp, ots, otc, aot = ctx
                for sub in range(2):
                    nc.tensor.matmul(
                        ots[sub],
                        lhsT=v_sb[:, kc, hp * 2 + sub, :],
                        rhs=est4[:, sub, half, :],
                        start=(kc == 0),
                        stop=(kc == kc_n - 1),
                    )
                if kc == kc_n - 1:
                    emit_otcopy(qg, hp, ots, otc)
                    if qg == qg_n - 1:
                        emit_norm(qg, otc, aot, hps=(hp,), tail=ots)
                        if hp == hp_n - 1:
                            emit_outproj(qg, aot)
                    elif hp == hp_n - 1:
                        pending_norm.append((qg, otc, aot))
                        pending_outproj.append((qg, aot, 0))

            for qg in range(qg_n):
                qs = slice(qg * 512, (qg + 1) * 512)
                aot = aot_pool.tile([P, kc2_n, 512], PDT, tag="aot", name=f"aot{qg}")
                otc = otc_pool.tile([dh + 1, 4, 512], F32, tag="otc", name=f"otc{qg}")
                for hp in range(hp_n):
                    ots = [
                        ot_ps.tile([dh + 1, 512], F32, tag="ot", name=f"ot{qg}_{hp}_{s}")
                        for s in range(2)
                    ]
                    ctx = (qg, hp, ots, otc, aot)
                    for kcp in range(kc_n // 2):
                        if qg == 0 and hp == 0:
                            emit_v(2 * kcp)
                            emit_v(2 * kcp + 1)
                        est4 = big.tile([P, 2, 2, 512], ADT, tag="big",
                                        name=f"est{qg}_{hp}_{kcp}")
                        # S^T chunks [128 keys, 512 queries] (K=64), sub0/
                        # sub1 into the two banks of one [128, 2, 512] psum
                        # tile; ONE exp op per (kcp, half) covers both subs.
                        for half in range(2):
                            kc = kcp * 2 + half
                            stp = st_ps.tile([P, 2, 512], F32, tag="st", name="stp")
                            for sub in range(2):
                                nc.tensor.matmul(
                                    stp[:, sub, :],
                                    lhsT=kT[sub * dh : (sub + 1) * dh, hp, kc * P : (kc + 1) * P],
                                    rhs=qT[sub * dh : (sub + 1) * dh, hp, qs],
                                    start=True,
                                    stop=True,
                                    tile_position=(sub * dh, 0),
                                )
                            dst = est4[:, :, half, :]
                            if dve_unit[kcp * 2 + half]:
                                # Schraudolph fast-exp on DVE: one mult-add
                                # into int16 bits, bitcast bf16.
                                nc.vector.tensor_scalar(
                                    out=dst.bitcast(I16),
                                    in0=stp,
                                    scalar1=EXP_A,
                                    scalar2=EXP_B,
                                    op0=mybir.AluOpType.mult,
                                    op1=mybir.AluOpType.add,
                                )
                            else:
                                nc.scalar.activation(
                                    out=dst,
                                    in_=stp,
                                    func=mybir.ActivationFunctionType.Exp,
                                )
                        for half in range(2):
                            ov_q.append((ctx, kcp * 2 + half, est4, half))
                        while len(ov_q) > ov_delay:
                            do_ov(*ov_q.pop(0))
                        if pending_norm and (hp == 0 and kcp >= 3 or hp == 1):
                            emit_norm(*pending_norm.pop(0))
                        if hp == 0 and kcp in (6, 7):
                            pop_mults(2)
                        elif hp == 1:
                            pop_mults(4)
                        if hp == 1 and kcp % 2 == 0 and pending_outproj:
                            pqg, paot, pj = pending_outproj[0]
                            emit_outproj_j(pqg, paot, pj)
                            if pj == 3:
                                pending_outproj.pop(0)
                            else:
                                pending_outproj[0] = (pqg, paot, pj + 1)
            for item in ov_q:
                do_ov(*item)
            pop_mults(4)
            for item in pending_norm:
                emit_norm(*item)
            pop_mults(4)
            for pqg, paot, pj in pending_outproj:
                for j in range(pj, 4):
                    emit_outproj_j(pqg, paot, j)
    nc.finalize()
    return nc


_NC_CACHE = {}


def _get_nc(mode="v3"):
    if mode not in _NC_CACHE:
        _NC_CACHE[mode] = build_attention_kernel_v4()
    return _NC_CACHE[mode]


def shard_inputs(x, gamma, w_qkv, w_out, mode="v3"):
    """FULL inputs -> list of 8 per-core input maps.

    Host-side prep (fp64): gamma+1 and the 1/sqrt(dh) attention scale are
    folded into w_qkv; the per-token rms scale rs = sqrt(d)/||x_t|| is
    folded into xT.
    """
    import ml_dtypes

    pdt = ml_dtypes.bfloat16
    d = x.shape[-1]
    dq = w_out.shape[0] // 4
    scale = DH ** -0.5
    gp1 = gamma.astype(np.float64) + 1.0
    w = w_qkv.astype(np.float64) * gp1[:, None]
    w[:, :d] *= scale  # q columns also absorb the softmax scale
    xs = x.astype(np.float64)
    rs = (d ** 0.5) / np.maximum(np.linalg.norm(xs, axis=-1), 1e-12)  # [b, n]
    xn = xs * rs[:, :, None]  # rms-normalized x (gamma fold lives in w)
    in_maps = []
    for c in range(NCORES):
        bi, g = c // 4, c % 4
        cs = slice(g * dq, (g + 1) * dq)
        wqkv_s = np.concatenate(
            [w[:, cs], w[:, d:][:, cs], w[:, 2 * d:][:, cs]], axis=1
        )
        xt = xn[bi].T.astype(pdt)  # [d, n]
        nhv = 4
        xt_tiled = np.ascontiguousarray(
            xt.reshape(d // P, P, nhv, x.shape[1] // nhv).transpose(2, 1, 0, 3)
        )
        wq = wqkv_s.astype(pdt)  # [d, 3*dq]
        wq_tiled = np.ascontiguousarray(
            wq.reshape(d // P, P, 3 * dq).transpose(1, 0, 2)
        )
        wo = w_out[cs, :].astype(pdt)  # [dq, d]
        wo_tiled = np.ascontiguousarray(
            wo.reshape(dq // P, P, d).transpose(1, 0, 2)
        )
        in_maps.append(
            {
                "xT": xt_tiled,
                "wqkv": wq_tiled,
                "wout": wo_tiled,
            }
        )
    return in_maps


def unshard_outputs(results):
    """8 partial [N, D] outputs -> full [B, N, D] (sum head groups per batch)."""
    outs = [np.asarray(r["out"], dtype=np.float32) for r in results]
    return np.stack(
        [
            outs[0] + outs[1] + outs[2] + outs[3],
            outs[4] + outs[5] + outs[6] + outs[7],
        ]
    ).astype(np.float32)


def run(x, gamma, w_qkv, w_out, mode="v3", **spmd_kwargs):
    nc = _get_nc(mode)
    in_maps = shard_inputs(x, gamma, w_qkv, w_out, mode)
    res = run_bass_kernel_spmd(nc, in_maps, list(range(NCORES)), **spmd_kwargs)
    return unshard_outputs(res.results), res


def kernel(x, gamma, w_qkv, w_out):
    out, _ = run(
        np.asarray(x), np.asarray(gamma), np.asarray(w_qkv), np.asarray(w_out)
    )
    return out
